# revision 1
# baseline (speedup 1.0000x reference)
"""Multi-head attention (causal, per-head projections) on 8 trn2 NeuronCores.

Sharding: core c = (batch b = c//2, head-quad = c%2). Each core computes 4
heads over all 2048 queries of its batch (identical static causal structure
on every core -> one SPMD program). A per-pair 2-core AllGather per query
window exchanges the per-head outputs (headsT) so both cores run the final
output Linear; the host keeps each core's query-half.

All compute in transposed-activation layout with float32r matmuls:
  X^T tiles (PE transpose) -> qT/kT = W.T @ X^T, v natural = (X^T chunks).T @ Wv
  scoresT[k, q] = kT.T @ qT   (k on partitions -> softmax sum via matmul)
  attnT = exp(scoresT/8)      (ACT, causal block-sliced; padding folded into v)
  ctxT_aug = [v*keep | keep].T @ attnT  (row 64 = softmax denominators)
  headsT = Wh.T @ (ctxT * bcast(1/rowsum))
  out = sum_h headsT_h.T @ Wo_h + bo
"""

import numpy as np

import concourse.bass as bass
import concourse.tile as tile
from concourse import bacc, mybir
from concourse import bass_utils

B, S, D, H, DK, DV = 4, 2048, 512, 8, 64, 64
HL = H // 2          # heads per core (4)
NW = S // 512        # 512-wide q windows (4)
NT = S // 128        # 128-row tiles (16)
F32 = mybir.dt.float32
F32R = mybir.dt.float32r
EXP = mybir.ActivationFunctionType.Exp


def build_program():
    nc = bacc.Bacc("TRN2", target_bir_lowering=False, debug=False, num_devices=8)

    def din(name, shape, dt=F32):
        return nc.dram_tensor(name, shape, dt, kind="ExternalInput").ap()

    xqT = din("xqT", [D, S], F32R)
    xkT = din("xkT", [D, S], F32R)
    xvT = din("xvT", [D, S], F32R)
    wq = din("wq", [128, 4, 256], F32R)
    wk = din("wk", [128, 4, 256], F32R)
    wv = din("wv", [128, 4, 256], F32R)
    wh = din("wh", [64, HL, 64], F32R)
    wo = din("wo", [128, 4, 512], F32R)
    bq = din("bq", [128, 2])
    bk = din("bk", [128, 2])
    bvb = din("bvb", [128, 256])
    bhb = din("bh", [64, HL])
    bob = din("bob", [128, 512])
    mask01 = din("mask01", [128, NT])   # 1.0 = keep key, 0.0 = padded-out key
    diagm = din("diagm", [128, 128])    # keep (row k, col q): q >= k
    ones1 = din("ones1", [1, 64], F32R)

    out = nc.dram_tensor("out", [S, D], F32, kind="ExternalOutput").ap()

    from contextlib import ExitStack

    with tile.TileContext(nc) as tc, ExitStack() as ctx:
        # ---- persistent SBUF ----
        pers = ctx.enter_context(tc.tile_pool(name="pers", bufs=1))
        qT_all = pers.tile([128, 2, S], F32R, tag="qT")
        kT_all = pers.tile([128, 2, S], F32R, tag="kT")
        v_sb = pers.tile([128, NT, HL * 65], F32R, tag="vsb")
        hrecv = pers.tile([128, 4, S], F32R, tag="hrecv")
        wq_sb = pers.tile([128, 4, 256], F32R, tag="wq")
        wk_sb = pers.tile([128, 4, 256], F32R, tag="wk")
        wv_sb = pers.tile([128, 4, 256], F32R, tag="wv")
        wh_sb = pers.tile([64, HL, 64], F32R, tag="wh")
        wo_sb = pers.tile([128, 4, 512], F32R, tag="wo")
        bq_sb = pers.tile([128, 2], F32, tag="bq")
        bk_sb = pers.tile([128, 2], F32, tag="bk")
        bvb_sb = pers.tile([128, 256], F32, tag="bvb")
        bh_sb = pers.tile([64, HL], F32, tag="bh")
        bob_sb = pers.tile([128, 512], F32, tag="bob")
        mask_sb = pers.tile([128, NT], F32, tag="mask")
        diagm_sb = pers.tile([128, 128], F32, tag="diagm")
        ones1_sb = pers.tile([1, 64], F32R, tag="ones1")

        for dst, src in [
            (wq_sb, wq), (wk_sb, wk), (wv_sb, wv), (wh_sb, wh), (wo_sb, wo),
            (bq_sb, bq), (bk_sb, bk), (bvb_sb, bvb), (bh_sb, bhb),
            (bob_sb, bob), (mask_sb, mask01), (diagm_sb, diagm),
            (ones1_sb, ones1),
        ]:
            nc.gpsimd.dma_start(out=dst, in_=src)

        # ---- DRAM bounce for the per-window heads exchange ----
        dram = ctx.enter_context(tc.tile_pool(name="dram", bufs=1, space="DRAM"))
        agin = [dram.tile([2, 64, 512], F32R, tag=f"agin{i}", name=f"agin{i}")
                for i in range(2 * NW)]
        agout = [dram.tile([2, 2, 64, 512], F32R, tag=f"agout{i}", name=f"agout{i}")
                 for i in range(2 * NW)]

        # ---- pools ----
        xtp = ctx.enter_context(tc.tile_pool(name="xtp", bufs=4))
        atp = ctx.enter_context(tc.tile_pool(name="atp", bufs=6))
        smp = ctx.enter_context(tc.tile_pool(name="smp", bufs=3))
        ostp = ctx.enter_context(tc.tile_pool(name="ostp", bufs=3))
        shr = ctx.enter_context(tc.tile_pool(name="shr", bufs=2, space="PSUM"))
        ppj = ctx.enter_context(tc.tile_pool(name="ppj", bufs=2, space="PSUM"))
        pcx = ctx.enter_context(tc.tile_pool(name="pcx", bufs=2, space="PSUM"))

        # ================= Phase 1: load X^T + projections =================
        for w in range(NW):
            xqTw = xtp.tile([128, 4, 512], F32R, tag="xT")
            xkTw = xtp.tile([128, 4, 512], F32R, tag="xT")
            xvTw = xtp.tile([128, 4, 512], F32R, tag="xT")
            for si, (src, dstT) in enumerate(((xqT, xqTw), (xkT, xkTw), (xvT, xvTw))):
                for dc in range(4):
                    eng = nc.sync if (si * 4 + dc) % 2 == 0 else nc.scalar
                    eng.dma_start(
                        out=dstT[:, dc, :],
                        in_=src[dc * 128 : dc * 128 + 128, w * 512 : (w + 1) * 512])
            # qT / kT projections for this window of 512 sequence positions
            for xT, w_sb, b_sb, dst in ((xqTw, wq_sb, bq_sb, qT_all), (xkTw, wk_sb, bk_sb, kT_all)):
                for hc in range(2):
                    pq = ppj.tile([128, 512], F32, tag="pj")
                    for dc in range(4):
                        nc.tensor.matmul(pq, w_sb[:, dc, hc * 128 : hc * 128 + 128],
                                         xT[:, dc, :], start=(dc == 0), stop=(dc == 3))
                    nc.vector.tensor_scalar_add(
                        out=dst[:, hc, w * 512 : (w + 1) * 512], in0=pq,
                        scalar1=b_sb[:, hc : hc + 1])
            # v natural layout (+bias, x padding keep-mask), per-head 65-col groups
            for t in range(4):
                tt = 4 * w + t
                pv = ppj.tile([128, 512], F32, tag="pj")
                for dc in range(4):
                    nc.tensor.matmul(pv[:, 0:256], xvTw[:, dc, t * 128 : t * 128 + 128],
                                     wv_sb[:, dc, :], start=(dc == 0), stop=(dc == 3))
                vst = smp.tile([128, 256], F32, tag="vst")
                nc.vector.tensor_add(out=vst, in0=pv[:, 0:256], in1=bvb_sb)
                v4 = v_sb[:, tt, :].rearrange("p (h u) -> p h u", u=65)
                nc.vector.tensor_scalar_mul(
                    out=v4[:, :, 0:64],
                    in0=vst.rearrange("p (h u) -> p h u", u=64),
                    scalar1=mask_sb[:, tt : tt + 1])
                mcol = mask_sb[:, tt : tt + 1]
                mbc = bass.AP(tensor=mcol.tensor, offset=mcol.offset,
                              ap=[mcol.ap[0], [0, HL]])
                nc.vector.tensor_scalar_add(out=v4[:, :, 64], in0=mbc, scalar1=0.0)

        # ============ Phase 2: attention + per-window exchange + out ============
        def emit_attention(w):
            n = 4 * (w + 1)
            for hp in range(2):
                hA, hB = 2 * hp, 2 * hp + 1      # base partitions 0 / 64
                pctxA = pcx.tile([65, 512], F32, tag="ctx", name="pctxA")
                pctxB = pcx.tile([65, 512], F32, tag="ctx", name="pctxB")
                for c0 in range(0, n, 2):
                    tiles = []
                    for c in (c0, c0 + 1):
                        j = c - 4 * w
                        qlo = max(0, 128 * j)
                        ps2 = shr.tile([128, 1024], F32, tag="big", name="ps2")
                        at2 = atp.tile([128, 1024], F32R, tag="at", name="at2")
                        tiles.append((c, qlo, at2))
                        for hi in range(2):
                            nc.tensor.matmul(
                                ps2[:, hi * 512 : hi * 512 + 512],
                                kT_all[64 * hi : 64 * hi + 64, hp, c * 128 : c * 128 + 128],
                                qT_all[64 * hi : 64 * hi + 64, hp, w * 512 : (w + 1) * 512],
                                start=True, stop=True)
                        if j < 0:
                            nc.scalar.activation(out=at2, in_=ps2, func=EXP,
                                                 bias=0.0, scale=0.125)
                        else:
                            for hi in range(2):
                                lo = hi * 512 + qlo
                                hi_ = hi * 512 + 512
                                nc.scalar.activation(out=at2[:, lo:hi_], in_=ps2[:, lo:hi_],
                                                     func=EXP, bias=0.0, scale=0.125)
                                nc.vector.tensor_mul(
                                    out=at2[:, lo : lo + 128],
                                    in0=at2[:, lo : lo + 128], in1=diagm_sb)
                    for c, qlo, at2 in tiles:
                        for hi, pctx_, hh in ((0, pctxA, hA), (1, pctxB, hB)):
                            nc.tensor.matmul(
                                pctx_[:, qlo:512],
                                v_sb[:, c, hh * 65 : hh * 65 + 65],
                                at2[:, hi * 512 + qlo : hi * 512 + 512],
                                start=(c == 0), stop=(c == n - 1))
                # normalize + per-head Linear, stage for exchange
                for pctx_, hh in ((pctxA, hA), (pctxB, hB)):
                    rr = smp.tile([1, 512], F32, tag="rr", name="rr")
                    nc.vector.tensor_scalar_add(out=rr, in0=pctx_[64:65, :], scalar1=0.0)
                    rrec = smp.tile([1, 512], F32, tag="rrec", name="rrec")
                    nc.vector.reciprocal_approx_fast(out=rrec, in_=rr)
                    rrec2 = smp.tile([1, 512], F32R, tag="rrec2", name="rrec2")
                    nc.vector.tensor_scalar_add(out=rrec2, in0=rrec, scalar1=0.0)
                    prb = ppj.tile([128, 512], F32, tag="pj", name="prb")
                    nc.tensor.matmul(prb[0:64, :], ones1_sb, rrec2, start=True, stop=True)
                    rbc = smp.tile([64, 512], F32, tag="rbc", name="rbc")
                    nc.scalar.add(out=rbc, in_=prb[0:64, :], add=0.0)
                    ctxn = smp.tile([64, 512], F32R, tag="ctxn", name="ctxn")
                    nc.vector.tensor_mul(out=ctxn, in0=pctx_[0:64, :], in1=rbc)
                    ph = ppj.tile([128, 512], F32, tag="pj", name="ph")
                    nc.tensor.matmul(ph[0:64, :], wh_sb[:, hh, :], ctxn, start=True, stop=True)
                    hst = smp.tile([64, 512], F32R, tag="hst", name="hst")
                    nc.vector.tensor_scalar_add(out=hst, in0=ph[0:64, :],
                                                scalar1=bh_sb[:, hh : hh + 1])
                    nc.sync.dma_start(out=agin[2 * w + hp][hh - 2 * hp, :, :], in_=hst)
                # exchange this head-pair's window slice
                gi = 2 * w + hp
                nc.gpsimd.collective_compute(
                    "AllGather", mybir.AluOpType.bypass,
                    replica_groups=[[0, 1], [2, 3], [4, 5], [6, 7]],
                    ins=[agin[gi].opt()], outs=[agout[gi].opt()])
                for r in range(2):
                    for j in range(2):
                        hh = r * 4 + 2 * hp + j
                        nc.sync.dma_start(
                            out=hrecv[64 * (hh % 2) : 64 * (hh % 2) + 64, 2 * r + hp,
                                      w * 512 : (w + 1) * 512],
                            in_=agout[gi][r, j, :, :])

        def emit_out(w):
            for qs in range(4 * w, 4 * w + 4):
                po = ppj.tile([128, 512], F32, tag="pj", name="po")
                for grp in range(4):
                    nc.tensor.matmul(po, hrecv[:, grp, qs * 128 : qs * 128 + 128],
                                     wo_sb[:, grp, :], start=(grp == 0), stop=(grp == 3))
                ost = ostp.tile([128, 512], F32, tag="ost", name="ost")
                nc.vector.tensor_add(out=ost, in0=po, in1=bob_sb)
                nc.sync.dma_start(out=out[qs * 128 : qs * 128 + 128, :], in_=ost)

        emit_attention(0)
        emit_attention(1)
        emit_out(0)
        emit_attention(2)
        emit_out(1)
        emit_attention(3)
        emit_out(2)
        emit_out(3)

    nc.compile()
    return nc


_NC = None


def _get_nc():
    global _NC
    if _NC is None:
        _NC = build_program()
    return _NC


def make_core_inputs(Q, K, V, padding_mask, Wq, bq, Wk, bk, Wv, bv, Wh, bh, Wo, bo):
    """Shard the full problem inputs into 8 per-core input dicts."""
    f = np.float32
    diagm = np.triu(np.ones((128, 128), f))  # keep q >= k  (row=k, col=q)
    bob = np.broadcast_to(np.asarray(bo, f), (128, 512)).copy()
    wo_in = np.zeros((128, 4, 512), f)
    Wo = np.asarray(Wo, f)
    for hh in range(H):
        wo_in[64 * (hh % 2) : 64 * (hh % 2) + 64, hh // 2, :] = Wo[hh * 64 : (hh + 1) * 64, :]

    ins = []
    for c in range(8):
        b, quad = c // 2, c % 2
        hlo = quad * HL
        wq_c = np.ascontiguousarray(np.transpose(np.asarray(Wq, f)[hlo : hlo + HL], (1, 0, 2))
                                    ).reshape(D, HL * DK)
        wk_c = np.ascontiguousarray(np.transpose(np.asarray(Wk, f)[hlo : hlo + HL], (1, 0, 2))
                                    ).reshape(D, HL * DK)
        wv_c = np.ascontiguousarray(np.transpose(np.asarray(Wv, f)[hlo : hlo + HL], (1, 0, 2))
                                    ).reshape(D, HL * DV)
        bq_c = np.asarray(bq, f)[hlo : hlo + HL].reshape(-1)
        bk_c = np.asarray(bk, f)[hlo : hlo + HL].reshape(-1)
        bv_c = np.asarray(bv, f)[hlo : hlo + HL].reshape(-1)
        pm = np.asarray(padding_mask[b, 0])
        keep = np.where(pm, np.float32(0.0), np.float32(1.0)).astype(f)
        ins.append({
            "xqT": np.ascontiguousarray(np.asarray(Q, f)[b].T),
            "xkT": np.ascontiguousarray(np.asarray(K, f)[b].T),
            "xvT": np.ascontiguousarray(np.asarray(V, f)[b].T),
            "wq": np.ascontiguousarray(wq_c.reshape(4, 128, 256).transpose(1, 0, 2)),
            "wk": np.ascontiguousarray(wk_c.reshape(4, 128, 256).transpose(1, 0, 2)),
            "wv": np.ascontiguousarray(wv_c.reshape(4, 128, 256).transpose(1, 0, 2)),
            "wh": np.ascontiguousarray(np.transpose(np.asarray(Wh, f)[hlo : hlo + HL], (1, 0, 2))),
            "wo": wo_in,
            "bq": np.ascontiguousarray(bq_c.reshape(2, 128).T),
            "bk": np.ascontiguousarray(bk_c.reshape(2, 128).T),
            "bvb": np.broadcast_to(bv_c, (128, HL * DV)).copy(),
            "bh": np.ascontiguousarray(np.asarray(bh, f)[hlo : hlo + HL].T),
            "bob": bob,
            "mask01": np.ascontiguousarray(keep.reshape(NT, 128).T),
            "diagm": diagm,
            "ones1": np.ones((1, 64), f),
        })
    return ins


def run(inputs_list, **kw):
    nc = _get_nc()
    return bass_utils.run_bass_kernel_spmd(nc, inputs_list, core_ids=list(range(8)), **kw)


def kernel(Q, K, V, padding_mask, Wq, bq, Wk, bk, Wv, bv, Wh, bh, Wo, bo):
    ins = make_core_inputs(Q, K, V, padding_mask, Wq, bq, Wk, bk, Wv, bv, Wh, bh, Wo, bo)
    res = run(ins)
    out = np.empty((B, S, D), np.float32)
    for c in range(8):
        b, quad = c // 2, c % 2
        out[b, quad * 1024 : (quad + 1) * 1024] = res.results[c]["out"][quad * 1024 : (quad + 1) * 1024]
    return out

